# revision 5
# baseline (speedup 1.0000x reference)
"""Trainium2 Bass kernel for nn_AgeUGP_v2 (gnn_message_passing).

Reference pipeline:
  snp_h[b,n,f] = snp[b,n] * filters[f,n]
  gathered     = snp_h[:, snp_ids, :]
  per_gene     = segment_sum(gathered, node_seg)   # node_seg sorted
  sample_h     = per_gene.mean(-1)
  h1 = sample_h @ W1 ... tiny MLP tail

Algebraic collapse: the filter axis F is only averaged at the end, so
  sample_h[b,g] = sum_{i in seg g} snp[b, id_i] * fbar[id_i],
  fbar = mean(filters, axis=0).

Device strategy (8 NeuronCores, genes sharded across cores):
  - SNP axis padded to 64 chunks of 8192; 4 table passes.  In pass T the
    128 partitions hold (chunk, batch) tables of v = snp * fbar in bf16:
    partition p = 16g + 8h + b holds chunk C[T][g+8h], batch b, where C
    is a per-core host-chosen chunk placement that pairs heavy chunks
    with light ones (min-max merged bucket => smallest stream length J).
  - Table build is fused: snp (host-permuted bf16) DMAs straight into
    the table; a 1/8-valued mean+replicate PE matmul over the
    host-permuted bf16 filters produces fbar in PSUM per 512 columns;
    the Activation engine casts it to a bf16 SBUF block; DVE multiplies
    the table in place at 2x bf16 rate.
  - One gather pass per table: gpsimd indirect_copy pulls the nodes of
    both chunk halves in one gene-ordered stream (group g's shared index
    stream is applied to all 16 lanes; each node is valid on its half's
    8 lanes, junk elsewhere is excluded by the sel combine).  A DVE
    tensor_tensor_scan (bf16 in, fp32 state+out) forms prefix sums; a
    second indirect_copy extracts prefixes at the A-end and B-end gene
    boundaries; one adjacent-difference (bf16 out) gives per-(half,gene,
    batch) partials.
  - Per pass, PE matmuls against 0/1 lane-selection columns accumulate
    the valid-lane sums directly in a PSUM tile across all 4 passes
    (start on pass 0, stop on pass 3) - no SBUF accumulator.
  - PE matmul with the core's W1 shard (bf16, host-permuted, prefetched
    one group per pass) -> partial h1 [8, 1024]; host sums the 8
    partials and runs the tiny MLP tail (0.01% of FLOPs).
Emission is software-pipelined (scan_p ahead of table/gather p+1, Pool
runs gather(p+1) between gather(p) and extract(p)) so Pool stays busy
back-to-back; DVE/Act/PE/DMA all fit under Pool's ~22.4us per pass.
"""

import numpy as np

B = 8
N_SNPS = 500000
N_NODES = 2000000
N_GENES = 20000
N_FILT = 8
N_CORES = 8
BN_EPS = 1e-5

_P = 128
_NCHUNK = 64  # SNP chunks
_NTAB = 4  # table passes
_EPAD = 16


def make_cfg(n_snps, n_genes, n_cores, chunk, d1, J):
    snp_pad = _NCHUNK * chunk
    assert snp_pad >= n_snps
    assert J % 16 == 0
    gpc = n_genes // n_cores
    jt = -(-gpc // _P)
    gpad_ = jt * _P
    ns = gpad_ + gpc + 1  # boundaries: dummy + gpad A-ends + gpc B-ends
    nspad = -(-ns // _EPAD) * _EPAD
    return dict(
        n_snps=n_snps, snp_pad=snp_pad, chunk=chunk,
        n_genes=n_genes, n_cores=n_cores, gpc=gpc, gpad=jt * _P, jt=jt,
        d1=d1, J=J, ns=ns, nspad=nspad,
    )


def full_cfg(J):
    return make_cfg(N_SNPS, N_GENES, N_CORES, 8192, 1024, J)


# ---------------------------------------------------------------- device program
def build_program(cfg):
    import concourse.bass as bass
    import concourse.bacc as bacc
    import concourse.mybir as mybir
    import concourse.tile as tile

    fp32 = mybir.dt.float32
    bf16 = mybir.dt.bfloat16
    u16 = mybir.dt.uint16

    chunk, snp_pad = cfg["chunk"], cfg["snp_pad"]
    jt, d1, J = cfg["jt"], cfg["d1"], cfg["J"]
    gpad, nspad, gpc = cfg["gpad"], cfg["nspad"], cfg["gpc"]
    nd = gpad + gpc

    nc = bacc.Bacc(
        "TRN2", target_bir_lowering=False, debug=False, num_devices=cfg["n_cores"]
    )

    snp_in = nc.dram_tensor(
        "snp_perm", [_P, _NTAB * chunk], bf16, kind="ExternalInput"
    )
    filt_in = nc.dram_tensor(
        "filt_perm", [_P, _NTAB * chunk], bf16, kind="ExternalInput"
    )
    gidx_in = nc.dram_tensor(
        "gidx", [_P, _NTAB * (J // 16)], u16, kind="ExternalInput"
    )
    eidx_in = nc.dram_tensor(
        "eidx", [_P, _NTAB * (nspad // 16)], u16, kind="ExternalInput"
    )
    sel_in = nc.dram_tensor("sel", [_P, 16], bf16, kind="ExternalInput")
    route_in = nc.dram_tensor("mroute", [_P, _P], bf16, kind="ExternalInput")
    w1_in = nc.dram_tensor("w1c", [_P, jt * d1], bf16, kind="ExternalInput")
    h1_out = nc.dram_tensor("h1p", [B, d1], fp32, kind="ExternalOutput")

    rc = 512
    nblk = chunk // rc
    nhv = 4
    fhalf = chunk // nhv
    wgrp = 5 if jt % 5 == 0 else 1  # K-tiles per W1 load
    nwld = jt // wgrp

    with tile.TileContext(nc) as tc:
        with (
            tc.tile_pool(name="per", bufs=1) as perpool,
            tc.tile_pool(name="tab", bufs=2) as tabpool,
            tc.tile_pool(name="ft", bufs=2) as ftpool,
            tc.tile_pool(name="fb", bufs=3) as fbpool,
            tc.tile_pool(name="gs", bufs=2) as gspool,
            tc.tile_pool(name="qq", bufs=2) as qpool,
            tc.tile_pool(name="ex", bufs=1) as expool,
            tc.tile_pool(name="dd", bufs=1) as ddpool,
            tc.tile_pool(name="w1", bufs=2) as w1pool,
            tc.tile_pool(name="pr", bufs=3, space="PSUM") as prpool,
            tc.tile_pool(name="pst", bufs=1, space="PSUM") as pstpool,
            tc.tile_pool(name="psw", bufs=1, space="PSUM") as pswpool,
        ):
            route = perpool.tile([_P, _P], bf16, tag="route")
            nc.sync.dma_start(route[:], route_in.ap())
            sel = perpool.tile([_P, 16], bf16, tag="sel")
            nc.sync.dma_start(sel[:], sel_in.ap())
            zs = perpool.tile([_P, 1], fp32, tag="zs")
            nc.vector.memset(zs[:], 0.0)

            # adjacent-difference output; pad cols [nd, 2*gpad) stay zero
            dd = ddpool.tile([_P, 2 * gpad], bf16, tag="dd", name="dd")
            if 2 * gpad > nd:
                nc.vector.memset(dd[:, nd:], 0.0)

            # PSUM accumulator for sample_h partials [gene-tile, (t, b)]
            pst = pstpool.tile([_P, jt * B], fp32, tag="pst", name="pst")

            vtabs = {}

            def emit_table(T):
                vtab = tabpool.tile([_P, chunk], bf16, tag="vtab", name=f"vtab{T}")
                for hv in range(nhv):
                    nc.sync.dma_start(
                        vtab[:, hv * fhalf : (hv + 1) * fhalf],
                        snp_in.ap()[:, T * chunk + hv * fhalf :
                                    T * chunk + (hv + 1) * fhalf],
                    )
                    ft = ftpool.tile(
                        [_P, fhalf], bf16, tag="ftl", name=f"ftl{T}_{hv}"
                    )
                    nc.sync.dma_start(
                        ft[:],
                        filt_in.ap()[:, T * chunk + hv * fhalf :
                                     T * chunk + (hv + 1) * fhalf],
                    )
                    for blk in range(nblk // nhv):
                        pr = prpool.tile([_P, rc], fp32, tag="pr", name="pr")
                        nc.tensor.matmul(
                            pr[:], route[:], ft[:, blk * rc : (blk + 1) * rc],
                            start=True, stop=True,
                        )
                        fb = fbpool.tile([_P, rc], bf16, tag="fb", name="fb")
                        nc.scalar.copy(fb[:], pr[:])
                        ks = slice(hv * fhalf + blk * rc,
                                   hv * fhalf + (blk + 1) * rc)
                        nc.vector.tensor_mul(vtab[:, ks], vtab[:, ks], fb[:])
                vtabs[T] = vtab

            def emit_gidx_load(p):
                gidx = gspool.tile(
                    [_P, J // 16], u16, tag="gidx", name=f"gidx{p}"
                )
                nc.sync.dma_start(
                    gidx[:],
                    gidx_in.ap()[:, p * (J // 16) : (p + 1) * (J // 16)],
                )
                return gidx

            def emit_eidx_load(p):
                eidx = gspool.tile(
                    [_P, nspad // 16], u16, tag="eidx", name=f"eidx{p}"
                )
                nc.sync.dma_start(
                    eidx[:],
                    eidx_in.ap()[:, p * (nspad // 16) : (p + 1) * (nspad // 16)],
                )
                return eidx

            def emit_gather(p, gidx):
                gout = gspool.tile([_P, J], bf16, tag="gout", name=f"gout{p}")
                nc.gpsimd.indirect_copy(gout[:], vtabs[p][:], gidx[:], True)
                return gout

            def emit_scan(p, gout):
                q = qpool.tile([_P, J], fp32, tag="q", name=f"q{p}")
                zbc = bass.AP(zs.tensor, zs[:].offset, [zs[:].ap[0], [0, J]])
                nc.vector.tensor_tensor_scan(
                    q[:], zbc, gout[:], 0.0,
                    op0=mybir.AluOpType.add, op1=mybir.AluOpType.add,
                )
                return q

            def emit_tail(p, q, eidx):
                ex = expool.tile([_P, nspad], fp32, tag="ex", name=f"ex{p}")
                nc.gpsimd.indirect_copy(ex[:], q[:], eidx[:], True)
                # E = [Q0, A-ends (gpad, padded), B-ends (gpc)]; adjacent
                # diffs give ddA at [0,gpad) and ddB at [gpad, gpad+gpc)
                nc.vector.tensor_sub(dd[:, :gpad], ex[:, 1 : gpad + 1],
                                     ex[:, :gpad])
                for t in range(jt):
                    nc.tensor.matmul(
                        pst[:, t * B : (t + 1) * B],
                        dd[:, t * _P : (t + 1) * _P],
                        sel[:, :8],
                        start=(p == 0), stop=False,
                    )
                nc.vector.tensor_sub(dd[:, gpad : nd], ex[:, gpad + 1 : nd + 1],
                                     ex[:, gpad : nd])
                for t in range(jt):
                    nc.tensor.matmul(
                        pst[:, t * B : (t + 1) * B],
                        dd[:, gpad + t * _P : gpad + (t + 1) * _P],
                        sel[:, 8:],
                        start=False, stop=(p == _NTAB - 1),
                    )

            w1ts = []

            def emit_w1_load(jg):
                w1t = w1pool.tile([_P, wgrp * d1], bf16, tag="w1t",
                                  name=f"w1t{jg}")
                nc.sync.dma_start(
                    w1t[:],
                    w1_in.ap()[:, jg * wgrp * d1 : (jg + 1) * wgrp * d1],
                )
                w1ts.append(w1t)

            # ---- software-pipelined emission ------------------------------
            gidxs = {0: emit_gidx_load(0)}
            eidxs = {0: emit_eidx_load(0)}
            emit_table(0)
            gouts = {0: emit_gather(0, gidxs.pop(0))}
            qs = {}
            for p in range(_NTAB):
                qs[p] = emit_scan(p, gouts.pop(p))
                if p + 1 < _NTAB:
                    gidxs[p + 1] = emit_gidx_load(p + 1)
                    eidxs[p + 1] = emit_eidx_load(p + 1)
                    emit_table(p + 1)
                    gouts[p + 1] = emit_gather(p + 1, gidxs.pop(p + 1))
                emit_tail(p, qs.pop(p), eidxs.pop(p))
                if p < nwld:
                    emit_w1_load(p)

            shb = perpool.tile([_P, jt * B], bf16, tag="shb")
            nc.vector.tensor_copy(shb[:], pst[:])

            # ---- W1 matmul: accumulate over jt K-tiles --------------------
            n_half = min(512, d1)
            n_banks = -(-d1 // n_half)
            pss = []
            for nb in range(n_banks):
                psw = pswpool.tile([_P, n_half], fp32, tag=f"ps{nb}",
                                   name=f"ps{nb}")
                pss.append(psw)
            for jg in range(nwld):
                w1t = w1ts[jg]
                for jl in range(wgrp):
                    j = jg * wgrp + jl
                    lhsT = shb[:, j * B : (j + 1) * B]
                    for nb in range(n_banks):
                        nc.tensor.matmul(
                            pss[nb][:B, :],
                            lhsT,
                            w1t[:, jl * d1 + nb * n_half : jl * d1 + (nb + 1) * n_half],
                            start=(j == 0),
                            stop=(j == jt - 1),
                        )

            h1 = perpool.tile([B, d1], fp32, tag="h1")
            for nb in range(n_banks):
                nc.vector.tensor_copy(
                    h1[:, nb * n_half : (nb + 1) * n_half], pss[nb][:B, :]
                )
            nc.sync.dma_start(h1_out.ap(), h1[:])

    nc.compile()
    return nc


# ---------------------------------------------------------------- host side
def _wrap16(streams, dtype):
    """[8, J] per-group streams -> [128, J//16] wrapped-16 layout."""
    ngrp, J = streams.shape
    assert ngrp == 8 and J % 16 == 0
    out = np.zeros((_P, J // 16), dtype)
    for g in range(8):
        out[g * 16 : (g + 1) * 16, :] = streams[g].reshape(J // 16, 16).T
    return out


def _chunk_placement(snp_ids, node_seg, chunk):
    """Per-core chunk->slot placement balancing merged A+B bucket sizes.

    Returns (J, [C_0..C_7]) where C_c[T][j] is the chunk held by slot j
    (j = g + 8h) of pass T on core c.
    """
    ids = np.asarray(snp_ids).astype(np.int64)
    seg = np.asarray(node_seg).astype(np.int64)
    gpc = N_GENES // N_CORES
    gene_starts = np.searchsorted(seg, np.arange(0, N_GENES + 1, gpc))
    Cs = []
    mx = 0
    for c in range(N_CORES):
        lo, hi = gene_starts[c], gene_starts[c + 1]
        cnt = np.bincount(ids[lo:hi] // chunk, minlength=_NCHUNK)
        order = np.argsort(cnt)[::-1]  # heavy..light
        C = np.zeros((_NTAB, 16), np.int64)
        for i in range(32):
            a, b = order[i], order[63 - i]
            T, g = i % _NTAB, i // _NTAB % 8
            C[T][g] = a
            C[T][8 + g] = b
            mx = max(mx, int(cnt[a] + cnt[b]))
        Cs.append(C)
    J = -(-(mx + 1) // 16) * 16
    assert J <= 65535, f"pass stream length {J} exceeds uint16 index range"
    return J, Cs


def prep_inputs(cfg, snp, snp_ids, node_seg, filters, W1, Cs):
    """Index/metadata preprocessing + zero-padding + pure layout permutation
    and bf16 casting; all value computation happens on device."""
    import ml_dtypes

    snp_pad_n, chunk = cfg["snp_pad"], cfg["chunk"]
    gpc, gpad, d1 = cfg["gpc"], cfg["gpad"], cfg["d1"]
    n_genes, n_snps = cfg["n_genes"], cfg["n_snps"]
    J, nspad = cfg["J"], cfg["nspad"]
    n_cores = cfg["n_cores"]
    bf = ml_dtypes.bfloat16

    snp_p = np.zeros((B, snp_pad_n), bf)
    snp_p[:, :n_snps] = np.asarray(snp, np.float32).astype(bf)
    filt_p = np.zeros((N_FILT, snp_pad_n), bf)
    filt_p[:, :n_snps] = np.asarray(filters, np.float32).astype(bf)
    snp_ch = snp_p.reshape(B, _NCHUNK, chunk)
    filt_ch = filt_p.reshape(N_FILT, _NCHUNK, chunk)

    # mean+replicate routing: out[m, j] = (1/8) sum_r filt_perm[8*slot(m)+r, j]
    # where slot(m) = m//16 + 8*((m%16)//8)
    mroute = np.zeros((_P, _P), bf)
    for mm in range(_P):
        spt = mm // 16 + 8 * ((mm % 16) // 8)
        mroute[spt * 8 : spt * 8 + 8, mm] = 1.0 / N_FILT

    sel = np.zeros((_P, 16), bf)
    for p in range(_P):
        sel[p, p % 16] = 1.0

    ids = np.asarray(snp_ids).astype(np.int64)
    seg = np.asarray(node_seg).astype(np.int64)
    gene_starts = np.searchsorted(seg, np.arange(0, n_genes + 1))
    node_chunk = ids // chunk
    node_lidx = (ids % chunk).astype(np.uint16)

    W1f = np.asarray(W1, np.float32)
    per_core = []
    for c in range(n_cores):
        C = Cs[c]
        # snp_perm: row p=16g+8h+b, pass-T cols hold snp[b, C[T][g+8h]-chunk]
        # filt_perm: row q holds filters[q%8, C[T][q//8]-chunk]
        snp_perm = np.empty((_P, _NTAB * chunk), bf)
        filt_perm = np.empty((_P, _NTAB * chunk), bf)
        for T in range(_NTAB):
            for g in range(8):
                for h in range(2):
                    ch = C[T][g + 8 * h]
                    rows = slice(16 * g + 8 * h, 16 * g + 8 * h + 8)
                    snp_perm[rows, T * chunk : (T + 1) * chunk] = snp_ch[:, ch, :]
            for j in range(16):
                ch = C[T][j]
                filt_perm[8 * j : 8 * j + 8, T * chunk : (T + 1) * chunk] = (
                    filt_ch[:, ch, :]
                )

        lo, hi = gene_starts[c * gpc], gene_starts[(c + 1) * gpc]
        cid_chunk = node_chunk[lo:hi]
        cid_lidx = node_lidx[lo:hi]
        cid_gene = seg[lo:hi] - c * gpc  # local gene, sorted ascending

        gidx = np.zeros((_NTAB, 8, J), np.uint16)
        eidx = np.zeros((_NTAB, 8, nspad), np.uint16)
        for T in range(_NTAB):
            for g_ in range(8):
                chA, chB = C[T][g_], C[T][8 + g_]
                mA = cid_chunk == chA
                mB = cid_chunk == chB
                lidxA, lgeneA = cid_lidx[mA], cid_gene[mA]
                lidxB, lgeneB = cid_lidx[mB], cid_gene[mB]
                cntA, cntB = len(lidxA), len(lidxB)
                assert cntA + cntB + 1 <= J, f"bucket {cntA+cntB} exceeds J={J}"
                # merged stream: [dummy, chunk-A nodes by gene, chunk-B nodes]
                gidx[T, g_, 1 : 1 + cntA] = lidxA
                gidx[T, g_, 1 + cntA : 1 + cntA + cntB] = lidxB
                # boundary positions: [0, A-ends (gpad, pad=end-of-A), B-ends]
                FA = np.searchsorted(lgeneA, np.arange(1, gpc + 1))
                FB = cntA + np.searchsorted(lgeneB, np.arange(1, gpc + 1))
                pos = np.zeros(nspad, np.int64)
                pos[1 : 1 + gpc] = FA
                pos[1 + gpc : 1 + gpad] = FA[-1] if gpc else 0
                pos[1 + gpad : 1 + gpad + gpc] = FB
                pos[1 + gpad + gpc :] = FB[-1]
                eidx[T, g_] = pos.astype(np.uint16)

        w1c = np.zeros((gpad, d1), np.float32)
        w1c[:gpc] = W1f[c * gpc : (c + 1) * gpc]
        jt_ = gpad // _P
        w1perm = np.ascontiguousarray(
            w1c.reshape(jt_, _P, d1).transpose(1, 0, 2).reshape(_P, jt_ * d1)
        ).astype(bf)
        gidx_all = np.concatenate(
            [_wrap16(gidx[p], np.uint16) for p in range(_NTAB)], axis=1
        )
        eidx_all = np.concatenate(
            [_wrap16(eidx[p], np.uint16) for p in range(_NTAB)], axis=1
        )
        core_map = dict(
            snp_perm=snp_perm, filt_perm=filt_perm, sel=sel, w1c=w1perm,
            mroute=mroute, gidx=gidx_all, eidx=eidx_all,
        )
        per_core.append(core_map)
    return per_core


def host_tail(h1_sum, b1, g1, be1, W2, b2, g2, be2, W3, b3, g3, be3,
              Wh1, bh1, gh, beh, Wh2, bh2):
    def bn(x, g, be):
        return x * (g / np.sqrt(np.float32(1.0 + BN_EPS))) + be

    relu = lambda x: np.maximum(x, np.float32(0.0))
    h = relu(bn(h1_sum + b1, g1, be1))
    h = relu(bn(h @ W2 + b2, g2, be2))
    feat = relu(bn(h @ W3 + b3, g3, be3))
    m = relu(bn(feat[:, :15] @ Wh1 + bh1, gh, beh))
    return (m @ Wh2 + bh2).astype(np.float32)


_CACHE = {}


def kernel(snp, snp_ids, node_seg, filters, W1, b1, g1, be1, W2, b2, g2, be2,
           W3, b3, g3, be3, Wh1, bh1, gh, beh, Wh2, bh2):
    from concourse import bass_utils

    J, Cs = _chunk_placement(snp_ids, node_seg, 8192)
    cfg = full_cfg(J)

    key = ("full", J)
    if key not in _CACHE:
        _CACHE[key] = build_program(cfg)
    nc = _CACHE[key]

    in_maps = prep_inputs(cfg, snp, snp_ids, node_seg, filters, W1, Cs)
    res = bass_utils.run_bass_kernel_spmd(
        nc, in_maps, core_ids=list(range(cfg["n_cores"]))
    )
    h1_sum = np.zeros((B, cfg["d1"]), np.float32)
    for c in range(cfg["n_cores"]):
        h1_sum += res.results[c]["h1p"]

    f32 = lambda x: np.asarray(x, np.float32)
    return host_tail(h1_sum, f32(b1), f32(g1), f32(be1), f32(W2), f32(b2),
                     f32(g2), f32(be2), f32(W3), f32(b3), f32(g3), f32(be3),
                     f32(Wh1), f32(bh1), f32(gh), f32(beh), f32(Wh2), f32(bh2))


# revision 11
# speedup vs baseline: 1.0553x; 1.0553x over previous
"""Trainium2 Bass kernel for nn_AgeUGP_v2 (gnn_message_passing).

Reference pipeline:
  snp_h[b,n,f] = snp[b,n] * filters[f,n]
  gathered     = snp_h[:, snp_ids, :]
  per_gene     = segment_sum(gathered, node_seg)   # node_seg sorted
  sample_h     = per_gene.mean(-1)
  h1 = sample_h @ W1 ... tiny MLP tail

Algebraic collapse: the filter axis F is only averaged at the end, so
  sample_h[b,g] = sum_{i in seg g} snp[b, id_i] * fbar[id_i],
  fbar = mean(filters, axis=0).

Device strategy (8 NeuronCores, genes sharded across cores):
  - SNP axis padded to 64 chunks of 8192; 4 table passes.  In pass T the
    128 partitions hold (chunk, batch) tables of v = snp * fbar in bf16:
    partition p = 16g + 8h + b holds chunk C[T][g+8h], batch b, where C
    is a per-core host-chosen chunk placement that pairs heavy chunks
    with light ones (min-max merged bucket => smallest stream length J).
  - Table build is fused: snp (host-permuted bf16) DMAs straight into
    the table; a 1/8-valued mean+replicate PE matmul over the
    host-permuted bf16 filters produces fbar in PSUM per 512 columns;
    the Activation engine casts it to a bf16 SBUF block; DVE multiplies
    the table in place at 2x bf16 rate.
  - One gather pass per table: gpsimd indirect_copy pulls the nodes of
    both chunk halves in one gene-ordered stream (group g's shared index
    stream is applied to all 16 lanes; each node is valid on its half's
    8 lanes, junk elsewhere is excluded by the sel combine).  A DVE
    tensor_tensor_scan (bf16 in, fp32 state+out) forms prefix sums; a
    second indirect_copy extracts prefixes at the A-end and B-end gene
    boundaries; one adjacent-difference (bf16 out) gives per-(half,gene,
    batch) partials.
  - Per pass, PE matmuls against 0/1 lane-selection columns accumulate
    the valid-lane sums directly in a PSUM tile across all 4 passes
    (start on pass 0, stop on pass 3) - no SBUF accumulator.
  - PE matmul with the core's W1 shard (bf16, host-permuted, prefetched
    one group per pass) -> partial h1 [8, 1024]; host sums the 8
    partials and runs the tiny MLP tail (0.01% of FLOPs).
Emission is software-pipelined (scan_p ahead of table/gather p+1, Pool
runs gather(p+1) between gather(p) and extract(p)) so Pool stays busy
back-to-back; DVE/Act/PE/DMA all fit under Pool's ~22.4us per pass.
"""

import numpy as np

B = 8
N_SNPS = 500000
N_NODES = 2000000
N_GENES = 20000
N_FILT = 8
N_CORES = 8
BN_EPS = 1e-5

_P = 128
_NCHUNK = 64  # SNP chunks
_NTAB = 4  # table passes
_EPAD = 16


def make_cfg(n_snps, n_genes, n_cores, chunk, d1, J):
    snp_pad = _NCHUNK * chunk
    assert snp_pad >= n_snps
    assert J % 16 == 0
    gpc = n_genes // n_cores
    jt = -(-gpc // _P)
    gpad_ = jt * _P
    ns = gpad_ + gpc + 1  # boundaries: dummy + gpad A-ends + gpc B-ends
    nspad = -(-ns // _EPAD) * _EPAD
    return dict(
        n_snps=n_snps, snp_pad=snp_pad, chunk=chunk,
        n_genes=n_genes, n_cores=n_cores, gpc=gpc, gpad=jt * _P, jt=jt,
        d1=d1, J=J, ns=ns, nspad=nspad,
    )


def full_cfg(J):
    return make_cfg(N_SNPS, N_GENES, N_CORES, 8192, 1024, J)


# ---------------------------------------------------------------- device program
def build_program(cfg):
    import concourse.bass as bass
    import concourse.bacc as bacc
    import concourse.mybir as mybir
    import concourse.tile as tile

    fp32 = mybir.dt.float32
    bf16 = mybir.dt.bfloat16
    u16 = mybir.dt.uint16

    chunk, snp_pad = cfg["chunk"], cfg["snp_pad"]
    jt, d1, J = cfg["jt"], cfg["d1"], cfg["J"]
    gpad, nspad, gpc = cfg["gpad"], cfg["nspad"], cfg["gpc"]
    nd = gpad + gpc

    nc = bacc.Bacc(
        "TRN2", target_bir_lowering=False, debug=False, num_devices=cfg["n_cores"]
    )

    snp_in = nc.dram_tensor(
        "snp_perm", [_P, _NTAB * chunk], bf16, kind="ExternalInput"
    )
    filt_in = nc.dram_tensor(
        "filt_perm", [_P, _NTAB * chunk], bf16, kind="ExternalInput"
    )
    gidx_in = nc.dram_tensor(
        "gidx", [_P, _NTAB * (J // 16)], u16, kind="ExternalInput"
    )
    eidx_in = nc.dram_tensor(
        "eidx", [_P, _NTAB * (nspad // 16)], u16, kind="ExternalInput"
    )
    sel_in = nc.dram_tensor("sel", [_P, 16], bf16, kind="ExternalInput")
    route_in = nc.dram_tensor("mroute", [_P, _P], bf16, kind="ExternalInput")
    w1_in = nc.dram_tensor("w1c", [_P, jt * d1], bf16, kind="ExternalInput")
    h1_out = nc.dram_tensor("h1p", [B, d1], fp32, kind="ExternalOutput")

    rc = 512
    nblk = chunk // rc
    nhv = 4
    fhalf = chunk // nhv
    wgrp = 5 if jt % 5 == 0 else 1  # K-tiles per W1 load
    nwld = jt // wgrp

    with tile.TileContext(nc) as tc:
        with (
            tc.tile_pool(name="per", bufs=1) as perpool,
            tc.tile_pool(name="tab", bufs=2) as tabpool,
            tc.tile_pool(name="ft", bufs=2) as ftpool,
            tc.tile_pool(name="fb", bufs=3) as fbpool,
            tc.tile_pool(name="gs", bufs=2) as gspool,
            tc.tile_pool(name="qq", bufs=2) as qpool,
            tc.tile_pool(name="ex", bufs=1) as expool,
            tc.tile_pool(name="dd", bufs=1) as ddpool,
            tc.tile_pool(name="w1", bufs=3) as w1pool,
            tc.tile_pool(name="pr", bufs=3, space="PSUM") as prpool,
            tc.tile_pool(name="pst", bufs=1, space="PSUM") as pstpool,
            tc.tile_pool(name="psw", bufs=1, space="PSUM") as pswpool,
        ):
            route = perpool.tile([_P, _P], bf16, tag="route")
            nc.sync.dma_start(route[:], route_in.ap())
            sel = perpool.tile([_P, 16], bf16, tag="sel")
            nc.sync.dma_start(sel[:], sel_in.ap())
            zs = perpool.tile([_P, 1], fp32, tag="zs")
            nc.vector.memset(zs[:], 0.0)

            # adjacent-difference output; pad cols [nd, 2*gpad) stay zero
            dd = ddpool.tile([_P, 2 * gpad], bf16, tag="dd", name="dd")
            if 2 * gpad > nd:
                nc.vector.memset(dd[:, nd:], 0.0)

            # PSUM accumulator for sample_h partials [gene-tile, (t, b)]
            pst = pstpool.tile([_P, jt * B], fp32, tag="pst", name="pst")

            vtabs = {}

            def emit_table(T):
                vtab = tabpool.tile([_P, chunk], bf16, tag="vtab", name=f"vtab{T}")
                for hv in range(nhv):
                    ft = ftpool.tile(
                        [_P, fhalf], bf16, tag="ftl", name=f"ftl{T}_{hv}"
                    )
                    nc.sync.dma_start(
                        ft[:],
                        filt_in.ap()[:, T * chunk + hv * fhalf :
                                     T * chunk + (hv + 1) * fhalf],
                    )
                    nc.sync.dma_start(
                        vtab[:, hv * fhalf : (hv + 1) * fhalf],
                        snp_in.ap()[:, T * chunk + hv * fhalf :
                                    T * chunk + (hv + 1) * fhalf],
                    )
                    for blk in range(nblk // nhv):
                        pr = prpool.tile([_P, rc], fp32, tag="pr", name="pr")
                        nc.tensor.matmul(
                            pr[:], route[:], ft[:, blk * rc : (blk + 1) * rc],
                            start=True, stop=True,
                        )
                        fb = fbpool.tile([_P, rc], bf16, tag="fb", name="fb")
                        nc.scalar.copy(fb[:], pr[:])
                        ks = slice(hv * fhalf + blk * rc,
                                   hv * fhalf + (blk + 1) * rc)
                        nc.vector.tensor_mul(vtab[:, ks], vtab[:, ks], fb[:])
                vtabs[T] = vtab

            def emit_gidx_load(p):
                gidx = gspool.tile(
                    [_P, J // 16], u16, tag="gidx", name=f"gidx{p}"
                )
                nc.sync.dma_start(
                    gidx[:],
                    gidx_in.ap()[:, p * (J // 16) : (p + 1) * (J // 16)],
                )
                return gidx

            def emit_eidx_load(p):
                eidx = gspool.tile(
                    [_P, nspad // 16], u16, tag="eidx", name=f"eidx{p}"
                )
                nc.sync.dma_start(
                    eidx[:],
                    eidx_in.ap()[:, p * (nspad // 16) : (p + 1) * (nspad // 16)],
                )
                return eidx

            def emit_gather(p, gidx):
                gout = gspool.tile([_P, J], bf16, tag="gout", name=f"gout{p}")
                nc.gpsimd.indirect_copy(gout[:], vtabs[p][:], gidx[:], True)
                return gout

            def emit_scan(p, gout):
                q = qpool.tile([_P, J], fp32, tag="q", name=f"q{p}")
                zbc = bass.AP(zs.tensor, zs[:].offset, [zs[:].ap[0], [0, J]])
                nc.vector.tensor_tensor_scan(
                    q[:], zbc, gout[:], 0.0,
                    op0=mybir.AluOpType.add, op1=mybir.AluOpType.add,
                )
                return q

            def emit_extract(p, q, eidx):
                ex = expool.tile([_P, nspad], fp32, tag="ex", name=f"ex{p}")
                nc.gpsimd.indirect_copy(ex[:], q[:], eidx[:], True)
                return ex

            def emit_subs(p, ex):
                # E = [Q0, A-ends (gpad, padded), B-ends (gpc)]; adjacent
                # diffs give ddA at [0,gpad) and ddB at [gpad, gpad+gpc)
                nc.vector.tensor_sub(dd[:, :gpad], ex[:, 1 : gpad + 1],
                                     ex[:, :gpad])
                for t in range(jt):
                    nc.tensor.matmul(
                        pst[:, t * B : (t + 1) * B],
                        dd[:, t * _P : (t + 1) * _P],
                        sel[:, :8],
                        start=(p == 0), stop=False,
                    )
                nc.vector.tensor_sub(dd[:, gpad : nd], ex[:, gpad + 1 : nd + 1],
                                     ex[:, gpad : nd])
                for t in range(jt):
                    nc.tensor.matmul(
                        pst[:, t * B : (t + 1) * B],
                        dd[:, gpad + t * _P : gpad + (t + 1) * _P],
                        sel[:, 8:],
                        start=False, stop=(p == _NTAB - 1),
                    )

            w1ts = []

            def emit_w1_load(jg):
                w1t = w1pool.tile([_P, wgrp * d1], bf16, tag="w1t",
                                  name=f"w1t{jg}")
                nc.sync.dma_start(
                    w1t[:],
                    w1_in.ap()[:, jg * wgrp * d1 : (jg + 1) * wgrp * d1],
                )
                w1ts.append(w1t)

            # ---- software-pipelined emission ------------------------------
            # Pool order: g0, g1, e0, g2, e1, g3, e2, e3 (gather p+1 slots
            # between extract p-1 and extract p so the scan hides behind it).
            # DVE order per cycle: muls(p+1), scan(p), subs(p-1).
            gidxs = {0: emit_gidx_load(0)}
            eidxs = {0: emit_eidx_load(0)}
            emit_table(0)
            gouts = {0: emit_gather(0, gidxs.pop(0))}
            qs, exs = {}, {}
            for p in range(_NTAB):
                if p + 1 < _NTAB:
                    gidxs[p + 1] = emit_gidx_load(p + 1)
                    eidxs[p + 1] = emit_eidx_load(p + 1)
                    emit_table(p + 1)
                if p > 0:
                    exs[p - 1] = emit_extract(p - 1, qs.pop(p - 1),
                                              eidxs.pop(p - 1))
                qs[p] = emit_scan(p, gouts.pop(p))
                if p + 1 < _NTAB:
                    gouts[p + 1] = emit_gather(p + 1, gidxs.pop(p + 1))
                if p > 0:
                    emit_subs(p - 1, exs.pop(p - 1))
                if p < nwld:
                    emit_w1_load(p)
            pl = _NTAB - 1
            exs[pl] = emit_extract(pl, qs.pop(pl), eidxs.pop(pl))
            emit_subs(pl, exs.pop(pl))

            shb = perpool.tile([_P, jt * B], bf16, tag="shb")
            nc.vector.tensor_copy(shb[:], pst[:])

            # ---- W1 matmul: accumulate over jt K-tiles --------------------
            n_half = min(512, d1)
            n_banks = -(-d1 // n_half)
            pss = []
            for nb in range(n_banks):
                psw = pswpool.tile([_P, n_half], fp32, tag=f"ps{nb}",
                                   name=f"ps{nb}")
                pss.append(psw)
            for jg in range(nwld):
                w1t = w1ts[jg]
                for jl in range(wgrp):
                    j = jg * wgrp + jl
                    lhsT = shb[:, j * B : (j + 1) * B]
                    for nb in range(n_banks):
                        nc.tensor.matmul(
                            pss[nb][:B, :],
                            lhsT,
                            w1t[:, jl * d1 + nb * n_half : jl * d1 + (nb + 1) * n_half],
                            start=(j == 0),
                            stop=(j == jt - 1),
                        )

            h1 = perpool.tile([B, d1], fp32, tag="h1")
            for nb in range(n_banks):
                nc.scalar.copy(h1[:, nb * n_half : (nb + 1) * n_half],
                               pss[nb][:B, :])
            nc.sync.dma_start(h1_out.ap(), h1[:])

    nc.compile()
    return nc


# ---------------------------------------------------------------- host side
def _wrap16(streams, dtype):
    """[8, J] per-group streams -> [128, J//16] wrapped-16 layout."""
    ngrp, J = streams.shape
    assert ngrp == 8 and J % 16 == 0
    out = np.zeros((_P, J // 16), dtype)
    for g in range(8):
        out[g * 16 : (g + 1) * 16, :] = streams[g].reshape(J // 16, 16).T
    return out


def _chunk_placement(snp_ids, node_seg, chunk):
    """Per-core chunk->slot placement balancing merged A+B bucket sizes.

    Returns (J, [C_0..C_7]) where C_c[T][j] is the chunk held by slot j
    (j = g + 8h) of pass T on core c.
    """
    ids = np.asarray(snp_ids).astype(np.int64)
    seg = np.asarray(node_seg).astype(np.int64)
    gpc = N_GENES // N_CORES
    gene_starts = np.searchsorted(seg, np.arange(0, N_GENES + 1, gpc))
    Cs = []
    mx = 0
    for c in range(N_CORES):
        lo, hi = gene_starts[c], gene_starts[c + 1]
        cnt = np.bincount(ids[lo:hi] // chunk, minlength=_NCHUNK)
        order = np.argsort(cnt)[::-1]  # heavy..light
        C = np.zeros((_NTAB, 16), np.int64)
        for i in range(32):
            a, b = order[i], order[63 - i]
            T, g = i % _NTAB, i // _NTAB % 8
            C[T][g] = a
            C[T][8 + g] = b
            mx = max(mx, int(cnt[a] + cnt[b]))
        Cs.append(C)
    J = -(-(mx + 1) // 16) * 16
    assert J <= 65535, f"pass stream length {J} exceeds uint16 index range"
    return J, Cs


def prep_inputs(cfg, snp, snp_ids, node_seg, filters, W1, Cs):
    """Index/metadata preprocessing + zero-padding + pure layout permutation
    and bf16 casting; all value computation happens on device."""
    import ml_dtypes

    snp_pad_n, chunk = cfg["snp_pad"], cfg["chunk"]
    gpc, gpad, d1 = cfg["gpc"], cfg["gpad"], cfg["d1"]
    n_genes, n_snps = cfg["n_genes"], cfg["n_snps"]
    J, nspad = cfg["J"], cfg["nspad"]
    n_cores = cfg["n_cores"]
    bf = ml_dtypes.bfloat16

    snp_p = np.zeros((B, snp_pad_n), bf)
    snp_p[:, :n_snps] = np.asarray(snp, np.float32).astype(bf)
    filt_p = np.zeros((N_FILT, snp_pad_n), bf)
    filt_p[:, :n_snps] = np.asarray(filters, np.float32).astype(bf)
    snp_ch = snp_p.reshape(B, _NCHUNK, chunk)
    filt_ch = filt_p.reshape(N_FILT, _NCHUNK, chunk)

    # mean+replicate routing: out[m, j] = (1/8) sum_r filt_perm[8*slot(m)+r, j]
    # where slot(m) = m//16 + 8*((m%16)//8)
    mroute = np.zeros((_P, _P), bf)
    for mm in range(_P):
        spt = mm // 16 + 8 * ((mm % 16) // 8)
        mroute[spt * 8 : spt * 8 + 8, mm] = 1.0 / N_FILT

    sel = np.zeros((_P, 16), bf)
    for p in range(_P):
        sel[p, p % 16] = 1.0

    ids = np.asarray(snp_ids).astype(np.int64)
    seg = np.asarray(node_seg).astype(np.int64)
    gene_starts = np.searchsorted(seg, np.arange(0, n_genes + 1))
    node_chunk = ids // chunk
    node_lidx = (ids % chunk).astype(np.uint16)

    W1f = np.asarray(W1, np.float32)
    per_core = []
    for c in range(n_cores):
        C = Cs[c]
        # snp_perm: row p=16g+8h+b, pass-T cols hold snp[b, C[T][g+8h]-chunk]
        # filt_perm: row q holds filters[q%8, C[T][q//8]-chunk]
        snp_perm = np.empty((_P, _NTAB * chunk), bf)
        filt_perm = np.empty((_P, _NTAB * chunk), bf)
        for T in range(_NTAB):
            for g in range(8):
                for h in range(2):
                    ch = C[T][g + 8 * h]
                    rows = slice(16 * g + 8 * h, 16 * g + 8 * h + 8)
                    snp_perm[rows, T * chunk : (T + 1) * chunk] = snp_ch[:, ch, :]
            for j in range(16):
                ch = C[T][j]
                filt_perm[8 * j : 8 * j + 8, T * chunk : (T + 1) * chunk] = (
                    filt_ch[:, ch, :]
                )

        lo, hi = gene_starts[c * gpc], gene_starts[(c + 1) * gpc]
        cid_chunk = node_chunk[lo:hi]
        cid_lidx = node_lidx[lo:hi]
        cid_gene = seg[lo:hi] - c * gpc  # local gene, sorted ascending

        gidx = np.zeros((_NTAB, 8, J), np.uint16)
        eidx = np.zeros((_NTAB, 8, nspad), np.uint16)
        for T in range(_NTAB):
            for g_ in range(8):
                chA, chB = C[T][g_], C[T][8 + g_]
                mA = cid_chunk == chA
                mB = cid_chunk == chB
                lidxA, lgeneA = cid_lidx[mA], cid_gene[mA]
                lidxB, lgeneB = cid_lidx[mB], cid_gene[mB]
                cntA, cntB = len(lidxA), len(lidxB)
                assert cntA + cntB + 1 <= J, f"bucket {cntA+cntB} exceeds J={J}"
                # merged stream: [dummy, chunk-A nodes by gene, chunk-B nodes]
                gidx[T, g_, 1 : 1 + cntA] = lidxA
                gidx[T, g_, 1 + cntA : 1 + cntA + cntB] = lidxB
                # boundary positions: [0, A-ends (gpad, pad=end-of-A), B-ends]
                FA = np.searchsorted(lgeneA, np.arange(1, gpc + 1))
                FB = cntA + np.searchsorted(lgeneB, np.arange(1, gpc + 1))
                pos = np.zeros(nspad, np.int64)
                pos[1 : 1 + gpc] = FA
                pos[1 + gpc : 1 + gpad] = FA[-1] if gpc else 0
                pos[1 + gpad : 1 + gpad + gpc] = FB
                pos[1 + gpad + gpc :] = FB[-1]
                eidx[T, g_] = pos.astype(np.uint16)

        w1c = np.zeros((gpad, d1), np.float32)
        w1c[:gpc] = W1f[c * gpc : (c + 1) * gpc]
        jt_ = gpad // _P
        w1perm = np.ascontiguousarray(
            w1c.reshape(jt_, _P, d1).transpose(1, 0, 2).reshape(_P, jt_ * d1)
        ).astype(bf)
        gidx_all = np.concatenate(
            [_wrap16(gidx[p], np.uint16) for p in range(_NTAB)], axis=1
        )
        eidx_all = np.concatenate(
            [_wrap16(eidx[p], np.uint16) for p in range(_NTAB)], axis=1
        )
        core_map = dict(
            snp_perm=snp_perm, filt_perm=filt_perm, sel=sel, w1c=w1perm,
            mroute=mroute, gidx=gidx_all, eidx=eidx_all,
        )
        per_core.append(core_map)
    return per_core


def host_tail(h1_sum, b1, g1, be1, W2, b2, g2, be2, W3, b3, g3, be3,
              Wh1, bh1, gh, beh, Wh2, bh2):
    def bn(x, g, be):
        return x * (g / np.sqrt(np.float32(1.0 + BN_EPS))) + be

    relu = lambda x: np.maximum(x, np.float32(0.0))
    h = relu(bn(h1_sum + b1, g1, be1))
    h = relu(bn(h @ W2 + b2, g2, be2))
    feat = relu(bn(h @ W3 + b3, g3, be3))
    m = relu(bn(feat[:, :15] @ Wh1 + bh1, gh, beh))
    return (m @ Wh2 + bh2).astype(np.float32)


_CACHE = {}


def kernel(snp, snp_ids, node_seg, filters, W1, b1, g1, be1, W2, b2, g2, be2,
           W3, b3, g3, be3, Wh1, bh1, gh, beh, Wh2, bh2):
    from concourse import bass_utils

    J, Cs = _chunk_placement(snp_ids, node_seg, 8192)
    cfg = full_cfg(J)

    key = ("full", J)
    if key not in _CACHE:
        _CACHE[key] = build_program(cfg)
    nc = _CACHE[key]

    in_maps = prep_inputs(cfg, snp, snp_ids, node_seg, filters, W1, Cs)
    res = bass_utils.run_bass_kernel_spmd(
        nc, in_maps, core_ids=list(range(cfg["n_cores"]))
    )
    h1_sum = np.zeros((B, cfg["d1"]), np.float32)
    for c in range(cfg["n_cores"]):
        h1_sum += res.results[c]["h1p"]

    f32 = lambda x: np.asarray(x, np.float32)
    return host_tail(h1_sum, f32(b1), f32(g1), f32(be1), f32(W2), f32(b2),
                     f32(g2), f32(be2), f32(W3), f32(b3), f32(g3), f32(be3),
                     f32(Wh1), f32(bh1), f32(gh), f32(beh), f32(Wh2), f32(bh2))


# revision 13
# speedup vs baseline: 1.0941x; 1.0368x over previous
"""Trainium2 Bass kernel for nn_AgeUGP_v2 (gnn_message_passing).

Reference pipeline:
  snp_h[b,n,f] = snp[b,n] * filters[f,n]
  gathered     = snp_h[:, snp_ids, :]
  per_gene     = segment_sum(gathered, node_seg)   # node_seg sorted
  sample_h     = per_gene.mean(-1)
  h1 = sample_h @ W1 ... tiny MLP tail

Algebraic collapse: the filter axis F is only averaged at the end, so
  sample_h[b,g] = sum_{i in seg g} snp[b, id_i] * fbar[id_i],
  fbar = mean(filters, axis=0).

Device strategy (8 NeuronCores, genes sharded across cores):
  - SNP axis padded to 64 chunks of 8192; 4 table passes.  In pass T the
    128 partitions hold (chunk, batch) tables of v = snp * fbar in bf16:
    partition p = 16g + 8h + b holds chunk C[T][g+8h], batch b, where C
    is a per-core host-chosen chunk placement that pairs heavy chunks
    with light ones (min-max merged bucket => smallest stream length J).
  - Table build is fused: snp (host-permuted bf16) DMAs straight into
    the table; a 1/8-valued mean+replicate PE matmul over the
    host-permuted bf16 filters produces fbar in PSUM per 512 columns;
    the Activation engine casts it to a bf16 SBUF block; DVE multiplies
    the table in place at 2x bf16 rate.
  - One gather pass per table: gpsimd indirect_copy pulls the nodes of
    both chunk halves in one gene-ordered stream (group g's shared index
    stream is applied to all 16 lanes; each node is valid on its half's
    8 lanes, junk elsewhere is excluded by the sel combine).  A DVE
    tensor_tensor_scan (bf16 in, fp32 state+out) forms prefix sums; a
    second indirect_copy extracts prefixes at the A-end and B-end gene
    boundaries; one adjacent-difference (bf16 out) gives per-(half,gene,
    batch) partials.
  - Per pass, PE matmuls against 0/1 lane-selection columns accumulate
    the valid-lane sums directly in a PSUM tile across all 4 passes
    (start on pass 0, stop on pass 3) - no SBUF accumulator.
  - PE matmul with the core's W1 shard (bf16, host-permuted, prefetched
    one group per pass) -> partial h1 [8, 1024]; host sums the 8
    partials and runs the tiny MLP tail (0.01% of FLOPs).
Emission is software-pipelined (scan_p ahead of table/gather p+1, Pool
runs gather(p+1) between gather(p) and extract(p)) so Pool stays busy
back-to-back; DVE/Act/PE/DMA all fit under Pool's ~22.4us per pass.
"""

import numpy as np

B = 8
N_SNPS = 500000
N_NODES = 2000000
N_GENES = 20000
N_FILT = 8
N_CORES = 8
BN_EPS = 1e-5

_P = 128
_NCHUNK = 64  # SNP chunks
_NTAB = 4  # table passes
_EPAD = 16


def make_cfg(n_snps, n_genes, n_cores, chunk, d1, J):
    snp_pad = _NCHUNK * chunk
    assert snp_pad >= n_snps
    assert J % 16 == 0
    gpc = n_genes // n_cores
    jt = -(-gpc // _P)
    gpad_ = jt * _P
    ns = gpad_ + gpc + 1  # boundaries: dummy + gpad A-ends + gpc B-ends
    nspad = -(-ns // _EPAD) * _EPAD
    return dict(
        n_snps=n_snps, snp_pad=snp_pad, chunk=chunk,
        n_genes=n_genes, n_cores=n_cores, gpc=gpc, gpad=jt * _P, jt=jt,
        d1=d1, J=J, ns=ns, nspad=nspad,
    )


def full_cfg(J):
    return make_cfg(N_SNPS, N_GENES, N_CORES, 8192, 1024, J)


# ---------------------------------------------------------------- device program
def build_program(cfg):
    import concourse.bass as bass
    import concourse.bacc as bacc
    import concourse.mybir as mybir
    import concourse.tile as tile

    fp32 = mybir.dt.float32
    bf16 = mybir.dt.bfloat16
    u16 = mybir.dt.uint16

    chunk, snp_pad = cfg["chunk"], cfg["snp_pad"]
    jt, d1, J = cfg["jt"], cfg["d1"], cfg["J"]
    gpad, nspad, gpc = cfg["gpad"], cfg["nspad"], cfg["gpc"]
    nd = gpad + gpc

    nc = bacc.Bacc(
        "TRN2", target_bir_lowering=False, debug=False, num_devices=cfg["n_cores"]
    )

    snp_in = nc.dram_tensor(
        "snp_perm", [_P, _NTAB * chunk], bf16, kind="ExternalInput"
    )
    filt_in = nc.dram_tensor(
        "filt_perm", [_P, _NTAB * chunk], bf16, kind="ExternalInput"
    )
    gidx_in = nc.dram_tensor(
        "gidx", [_P, _NTAB * (J // 16)], u16, kind="ExternalInput"
    )
    eidx_in = nc.dram_tensor(
        "eidx", [_P, _NTAB * (nspad // 16)], u16, kind="ExternalInput"
    )
    sel_in = nc.dram_tensor("sel", [_P, 16], bf16, kind="ExternalInput")
    route_in = nc.dram_tensor("mroute", [_P, _P], bf16, kind="ExternalInput")
    w1_in = nc.dram_tensor("w1c", [_P, jt * d1], bf16, kind="ExternalInput")
    h1_out = nc.dram_tensor("h1p", [B, d1], fp32, kind="ExternalOutput")

    rc = 512
    nblk = chunk // rc
    nhv = 4
    fhalf = chunk // nhv
    wgrp = 5 if jt % 5 == 0 else 1  # K-tiles per W1 load
    nwld = jt // wgrp

    with tile.TileContext(nc) as tc:
        with (
            tc.tile_pool(name="per", bufs=1) as perpool,
            tc.tile_pool(name="tab", bufs=2) as tabpool,
            tc.tile_pool(name="ft", bufs=2) as ftpool,
            tc.tile_pool(name="fb", bufs=3) as fbpool,
            tc.tile_pool(name="gs", bufs=2) as gspool,
            tc.tile_pool(name="qq", bufs=2) as qpool,
            tc.tile_pool(name="ex", bufs=1) as expool,
            tc.tile_pool(name="dd", bufs=1) as ddpool,
            tc.tile_pool(name="w1", bufs=3) as w1pool,
            tc.tile_pool(name="pr", bufs=3, space="PSUM") as prpool,
            tc.tile_pool(name="pst", bufs=1, space="PSUM") as pstpool,
            tc.tile_pool(name="psw", bufs=1, space="PSUM") as pswpool,
        ):
            route = perpool.tile([_P, _P], bf16, tag="route")
            nc.sync.dma_start(route[:], route_in.ap())
            sel = perpool.tile([_P, 16], bf16, tag="sel")
            nc.sync.dma_start(sel[:], sel_in.ap())
            zs = perpool.tile([_P, 1], fp32, tag="zs")
            nc.vector.memset(zs[:], 0.0)

            # adjacent-difference output; pad cols [nd, 2*gpad) stay zero
            dd = ddpool.tile([_P, 2 * gpad], bf16, tag="dd", name="dd")
            if 2 * gpad > nd:
                nc.vector.memset(dd[:, nd:], 0.0)

            # PSUM accumulator for sample_h partials [gene-tile, (t, b)]
            pst = pstpool.tile([_P, jt * B], fp32, tag="pst", name="pst")

            vtabs = {}

            def emit_table(T):
                vtab = tabpool.tile([_P, chunk], bf16, tag="vtab", name=f"vtab{T}")
                for hv in range(nhv):
                    ft = ftpool.tile(
                        [_P, fhalf], bf16, tag="ftl", name=f"ftl{T}_{hv}"
                    )
                    nc.sync.dma_start(
                        ft[:],
                        filt_in.ap()[:, T * chunk + hv * fhalf :
                                     T * chunk + (hv + 1) * fhalf],
                    )
                    nc.sync.dma_start(
                        vtab[:, hv * fhalf : (hv + 1) * fhalf],
                        snp_in.ap()[:, T * chunk + hv * fhalf :
                                    T * chunk + (hv + 1) * fhalf],
                    )
                    for blk in range(nblk // nhv):
                        pr = prpool.tile([_P, rc], fp32, tag="pr", name="pr")
                        nc.tensor.matmul(
                            pr[:], route[:], ft[:, blk * rc : (blk + 1) * rc],
                            start=True, stop=True,
                        )
                        fb = fbpool.tile([_P, rc], bf16, tag="fb", name="fb")
                        nc.scalar.copy(fb[:], pr[:])
                        ks = slice(hv * fhalf + blk * rc,
                                   hv * fhalf + (blk + 1) * rc)
                        nc.vector.tensor_mul(vtab[:, ks], vtab[:, ks], fb[:])
                vtabs[T] = vtab

            def emit_gidx_load(p):
                gidx = gspool.tile(
                    [_P, J // 16], u16, tag="gidx", name=f"gidx{p}"
                )
                nc.sync.dma_start(
                    gidx[:],
                    gidx_in.ap()[:, p * (J // 16) : (p + 1) * (J // 16)],
                )
                return gidx

            def emit_eidx_load(p):
                eidx = gspool.tile(
                    [_P, nspad // 16], u16, tag="eidx", name=f"eidx{p}"
                )
                nc.sync.dma_start(
                    eidx[:],
                    eidx_in.ap()[:, p * (nspad // 16) : (p + 1) * (nspad // 16)],
                )
                return eidx

            gather_insts = {}

            def emit_gather(p, gidx):
                gout = gspool.tile([_P, J], bf16, tag="gout", name=f"gout{p}")
                gi = nc.gpsimd.indirect_copy(gout[:], vtabs[p][:], gidx[:], True)
                gather_insts[p] = gi
                return gout

            def emit_scan(p, gout):
                q = qpool.tile([_P, J], fp32, tag="q", name=f"q{p}")
                zbc = bass.AP(zs.tensor, zs[:].offset, [zs[:].ap[0], [0, J]])
                nc.vector.tensor_tensor_scan(
                    q[:], zbc, gout[:], 0.0,
                    op0=mybir.AluOpType.add, op1=mybir.AluOpType.add,
                )
                return q

            def emit_extract(p, q, eidx):
                import concourse.bass as _bass_mod

                ex = expool.tile([_P, nspad], fp32, tag="ex", name=f"ex{p}")
                ei = nc.gpsimd.indirect_copy(ex[:], q[:], eidx[:], True)
                # keep Pool busy: extract p must not preempt gather p+1
                if p + 1 in gather_insts:
                    _bass_mod._add_dep_helper(
                        ei.ins, gather_insts[p + 1].ins, sync=True,
                        reason="pipeline: extract after next gather",
                    )
                return ex

            def emit_subs(p, ex):
                # E = [Q0, A-ends (gpad, padded), B-ends (gpc)]; adjacent
                # diffs give ddA at [0,gpad) and ddB at [gpad, gpad+gpc)
                nc.vector.tensor_sub(dd[:, :gpad], ex[:, 1 : gpad + 1],
                                     ex[:, :gpad])
                for t in range(jt):
                    nc.tensor.matmul(
                        pst[:, t * B : (t + 1) * B],
                        dd[:, t * _P : (t + 1) * _P],
                        sel[:, :8],
                        start=(p == 0), stop=False,
                    )
                nc.vector.tensor_sub(dd[:, gpad : nd], ex[:, gpad + 1 : nd + 1],
                                     ex[:, gpad : nd])
                for t in range(jt):
                    nc.tensor.matmul(
                        pst[:, t * B : (t + 1) * B],
                        dd[:, gpad + t * _P : gpad + (t + 1) * _P],
                        sel[:, 8:],
                        start=False, stop=(p == _NTAB - 1),
                    )

            w1ts = []

            def emit_w1_load(jg):
                w1t = w1pool.tile([_P, wgrp * d1], bf16, tag="w1t",
                                  name=f"w1t{jg}")
                nc.sync.dma_start(
                    w1t[:],
                    w1_in.ap()[:, jg * wgrp * d1 : (jg + 1) * wgrp * d1],
                )
                w1ts.append(w1t)

            # ---- software-pipelined emission ------------------------------
            # Pool order: g0, g1, e0, g2, e1, g3, e2, e3 (gather p+1 slots
            # between extract p-1 and extract p so the scan hides behind it).
            # DVE order per cycle: muls(p+1), scan(p), subs(p-1).
            gidxs = {0: emit_gidx_load(0)}
            eidxs = {0: emit_eidx_load(0)}
            emit_table(0)
            gouts = {0: emit_gather(0, gidxs.pop(0))}
            qs, exs = {}, {}
            for p in range(_NTAB):
                if p + 1 < _NTAB:
                    gidxs[p + 1] = emit_gidx_load(p + 1)
                    eidxs[p + 1] = emit_eidx_load(p + 1)
                    emit_table(p + 1)
                if p > 0:
                    exs[p - 1] = emit_extract(p - 1, qs.pop(p - 1),
                                              eidxs.pop(p - 1))
                qs[p] = emit_scan(p, gouts.pop(p))
                if p + 1 < _NTAB:
                    gouts[p + 1] = emit_gather(p + 1, gidxs.pop(p + 1))
                if p > 0:
                    emit_subs(p - 1, exs.pop(p - 1))
                if p < nwld:
                    emit_w1_load(p)
            pl = _NTAB - 1
            exs[pl] = emit_extract(pl, qs.pop(pl), eidxs.pop(pl))
            emit_subs(pl, exs.pop(pl))

            shb = perpool.tile([_P, jt * B], bf16, tag="shb")
            nc.vector.tensor_copy(shb[:], pst[:])

            # ---- W1 matmul: accumulate over jt K-tiles --------------------
            n_half = min(512, d1)
            n_banks = -(-d1 // n_half)
            pss = []
            for nb in range(n_banks):
                psw = pswpool.tile([_P, n_half], fp32, tag=f"ps{nb}",
                                   name=f"ps{nb}")
                pss.append(psw)
            for jg in range(nwld):
                w1t = w1ts[jg]
                for jl in range(wgrp):
                    j = jg * wgrp + jl
                    lhsT = shb[:, j * B : (j + 1) * B]
                    for nb in range(n_banks):
                        nc.tensor.matmul(
                            pss[nb][:B, :],
                            lhsT,
                            w1t[:, jl * d1 + nb * n_half : jl * d1 + (nb + 1) * n_half],
                            start=(j == 0),
                            stop=(j == jt - 1),
                        )

            h1 = perpool.tile([B, d1], fp32, tag="h1")
            for nb in range(n_banks):
                nc.scalar.copy(h1[:, nb * n_half : (nb + 1) * n_half],
                               pss[nb][:B, :])
            nc.sync.dma_start(h1_out.ap(), h1[:])

    nc.compile()
    return nc


# ---------------------------------------------------------------- host side
def _wrap16(streams, dtype):
    """[8, J] per-group streams -> [128, J//16] wrapped-16 layout."""
    ngrp, J = streams.shape
    assert ngrp == 8 and J % 16 == 0
    out = np.zeros((_P, J // 16), dtype)
    for g in range(8):
        out[g * 16 : (g + 1) * 16, :] = streams[g].reshape(J // 16, 16).T
    return out


def _chunk_placement(snp_ids, node_seg, chunk):
    """Per-core chunk->slot placement balancing merged A+B bucket sizes.

    Returns (J, [C_0..C_7]) where C_c[T][j] is the chunk held by slot j
    (j = g + 8h) of pass T on core c.
    """
    ids = np.asarray(snp_ids).astype(np.int64)
    seg = np.asarray(node_seg).astype(np.int64)
    gpc = N_GENES // N_CORES
    gene_starts = np.searchsorted(seg, np.arange(0, N_GENES + 1, gpc))
    Cs = []
    mx = 0
    for c in range(N_CORES):
        lo, hi = gene_starts[c], gene_starts[c + 1]
        cnt = np.bincount(ids[lo:hi] // chunk, minlength=_NCHUNK)
        order = np.argsort(cnt)[::-1]  # heavy..light
        C = np.zeros((_NTAB, 16), np.int64)
        for i in range(32):
            a, b = order[i], order[63 - i]
            T, g = i % _NTAB, i // _NTAB % 8
            C[T][g] = a
            C[T][8 + g] = b
            mx = max(mx, int(cnt[a] + cnt[b]))
        Cs.append(C)
    J = -(-(mx + 1) // 16) * 16
    assert J <= 65535, f"pass stream length {J} exceeds uint16 index range"
    return J, Cs


def prep_inputs(cfg, snp, snp_ids, node_seg, filters, W1, Cs):
    """Index/metadata preprocessing + zero-padding + pure layout permutation
    and bf16 casting; all value computation happens on device."""
    import ml_dtypes

    snp_pad_n, chunk = cfg["snp_pad"], cfg["chunk"]
    gpc, gpad, d1 = cfg["gpc"], cfg["gpad"], cfg["d1"]
    n_genes, n_snps = cfg["n_genes"], cfg["n_snps"]
    J, nspad = cfg["J"], cfg["nspad"]
    n_cores = cfg["n_cores"]
    bf = ml_dtypes.bfloat16

    snp_p = np.zeros((B, snp_pad_n), bf)
    snp_p[:, :n_snps] = np.asarray(snp, np.float32).astype(bf)
    filt_p = np.zeros((N_FILT, snp_pad_n), bf)
    filt_p[:, :n_snps] = np.asarray(filters, np.float32).astype(bf)
    snp_ch = snp_p.reshape(B, _NCHUNK, chunk)
    filt_ch = filt_p.reshape(N_FILT, _NCHUNK, chunk)

    # mean+replicate routing: out[m, j] = (1/8) sum_r filt_perm[8*slot(m)+r, j]
    # where slot(m) = m//16 + 8*((m%16)//8)
    mroute = np.zeros((_P, _P), bf)
    for mm in range(_P):
        spt = mm // 16 + 8 * ((mm % 16) // 8)
        mroute[spt * 8 : spt * 8 + 8, mm] = 1.0 / N_FILT

    sel = np.zeros((_P, 16), bf)
    for p in range(_P):
        sel[p, p % 16] = 1.0

    ids = np.asarray(snp_ids).astype(np.int64)
    seg = np.asarray(node_seg).astype(np.int64)
    gene_starts = np.searchsorted(seg, np.arange(0, n_genes + 1))
    node_chunk = ids // chunk
    node_lidx = (ids % chunk).astype(np.uint16)

    W1f = np.asarray(W1, np.float32)
    per_core = []
    for c in range(n_cores):
        C = Cs[c]
        # snp_perm: row p=16g+8h+b, pass-T cols hold snp[b, C[T][g+8h]-chunk]
        # filt_perm: row q holds filters[q%8, C[T][q//8]-chunk]
        snp_perm = np.empty((_P, _NTAB * chunk), bf)
        filt_perm = np.empty((_P, _NTAB * chunk), bf)
        for T in range(_NTAB):
            for g in range(8):
                for h in range(2):
                    ch = C[T][g + 8 * h]
                    rows = slice(16 * g + 8 * h, 16 * g + 8 * h + 8)
                    snp_perm[rows, T * chunk : (T + 1) * chunk] = snp_ch[:, ch, :]
            for j in range(16):
                ch = C[T][j]
                filt_perm[8 * j : 8 * j + 8, T * chunk : (T + 1) * chunk] = (
                    filt_ch[:, ch, :]
                )

        lo, hi = gene_starts[c * gpc], gene_starts[(c + 1) * gpc]
        cid_chunk = node_chunk[lo:hi]
        cid_lidx = node_lidx[lo:hi]
        cid_gene = seg[lo:hi] - c * gpc  # local gene, sorted ascending

        gidx = np.zeros((_NTAB, 8, J), np.uint16)
        eidx = np.zeros((_NTAB, 8, nspad), np.uint16)
        for T in range(_NTAB):
            for g_ in range(8):
                chA, chB = C[T][g_], C[T][8 + g_]
                mA = cid_chunk == chA
                mB = cid_chunk == chB
                lidxA, lgeneA = cid_lidx[mA], cid_gene[mA]
                lidxB, lgeneB = cid_lidx[mB], cid_gene[mB]
                cntA, cntB = len(lidxA), len(lidxB)
                assert cntA + cntB + 1 <= J, f"bucket {cntA+cntB} exceeds J={J}"
                # merged stream: [dummy, chunk-A nodes by gene, chunk-B nodes]
                gidx[T, g_, 1 : 1 + cntA] = lidxA
                gidx[T, g_, 1 + cntA : 1 + cntA + cntB] = lidxB
                # boundary positions: [0, A-ends (gpad, pad=end-of-A), B-ends]
                FA = np.searchsorted(lgeneA, np.arange(1, gpc + 1))
                FB = cntA + np.searchsorted(lgeneB, np.arange(1, gpc + 1))
                pos = np.zeros(nspad, np.int64)
                pos[1 : 1 + gpc] = FA
                pos[1 + gpc : 1 + gpad] = FA[-1] if gpc else 0
                pos[1 + gpad : 1 + gpad + gpc] = FB
                pos[1 + gpad + gpc :] = FB[-1]
                eidx[T, g_] = pos.astype(np.uint16)

        w1c = np.zeros((gpad, d1), np.float32)
        w1c[:gpc] = W1f[c * gpc : (c + 1) * gpc]
        jt_ = gpad // _P
        w1perm = np.ascontiguousarray(
            w1c.reshape(jt_, _P, d1).transpose(1, 0, 2).reshape(_P, jt_ * d1)
        ).astype(bf)
        gidx_all = np.concatenate(
            [_wrap16(gidx[p], np.uint16) for p in range(_NTAB)], axis=1
        )
        eidx_all = np.concatenate(
            [_wrap16(eidx[p], np.uint16) for p in range(_NTAB)], axis=1
        )
        core_map = dict(
            snp_perm=snp_perm, filt_perm=filt_perm, sel=sel, w1c=w1perm,
            mroute=mroute, gidx=gidx_all, eidx=eidx_all,
        )
        per_core.append(core_map)
    return per_core


def host_tail(h1_sum, b1, g1, be1, W2, b2, g2, be2, W3, b3, g3, be3,
              Wh1, bh1, gh, beh, Wh2, bh2):
    def bn(x, g, be):
        return x * (g / np.sqrt(np.float32(1.0 + BN_EPS))) + be

    relu = lambda x: np.maximum(x, np.float32(0.0))
    h = relu(bn(h1_sum + b1, g1, be1))
    h = relu(bn(h @ W2 + b2, g2, be2))
    feat = relu(bn(h @ W3 + b3, g3, be3))
    m = relu(bn(feat[:, :15] @ Wh1 + bh1, gh, beh))
    return (m @ Wh2 + bh2).astype(np.float32)


_CACHE = {}


def kernel(snp, snp_ids, node_seg, filters, W1, b1, g1, be1, W2, b2, g2, be2,
           W3, b3, g3, be3, Wh1, bh1, gh, beh, Wh2, bh2):
    from concourse import bass_utils

    J, Cs = _chunk_placement(snp_ids, node_seg, 8192)
    cfg = full_cfg(J)

    key = ("full", J)
    if key not in _CACHE:
        _CACHE[key] = build_program(cfg)
    nc = _CACHE[key]

    in_maps = prep_inputs(cfg, snp, snp_ids, node_seg, filters, W1, Cs)
    res = bass_utils.run_bass_kernel_spmd(
        nc, in_maps, core_ids=list(range(cfg["n_cores"]))
    )
    h1_sum = np.zeros((B, cfg["d1"]), np.float32)
    for c in range(cfg["n_cores"]):
        h1_sum += res.results[c]["h1p"]

    f32 = lambda x: np.asarray(x, np.float32)
    return host_tail(h1_sum, f32(b1), f32(g1), f32(be1), f32(W2), f32(b2),
                     f32(g2), f32(be2), f32(W3), f32(b3), f32(g3), f32(be3),
                     f32(Wh1), f32(bh1), f32(gh), f32(beh), f32(Wh2), f32(bh2))


# revision 17
# speedup vs baseline: 1.1881x; 1.0859x over previous
"""Trainium2 Bass kernel for nn_AgeUGP_v2 (gnn_message_passing).

Reference pipeline:
  snp_h[b,n,f] = snp[b,n] * filters[f,n]
  gathered     = snp_h[:, snp_ids, :]
  per_gene     = segment_sum(gathered, node_seg)   # node_seg sorted
  sample_h     = per_gene.mean(-1)
  h1 = sample_h @ W1 ... tiny MLP tail

Algebraic collapse: the filter axis F is only averaged at the end, so
  sample_h[b,g] = sum_{i in seg g} snp[b, id_i] * fbar[id_i],
  fbar = mean(filters, axis=0).

Device strategy (8 NeuronCores, genes sharded across cores):
  - SNP axis padded to 64 chunks of 8192; 4 table passes.  In pass T the
    128 partitions hold (chunk, batch) tables of v = snp * fbar in bf16:
    partition p = 16g + 8h + b holds chunk C[T][g+8h], batch b, where C
    is a per-core host-chosen chunk placement that pairs heavy chunks
    with light ones (min-max merged bucket => smallest stream length J).
  - Table build is fused: snp (host-permuted bf16) DMAs straight into
    the table; a 1/8-valued mean+replicate PE matmul over the
    host-permuted bf16 filters produces fbar in PSUM per 512 columns;
    the Activation engine casts it to a bf16 SBUF block; DVE multiplies
    the table in place at 2x bf16 rate.
  - One gather pass per table: gpsimd indirect_copy pulls the nodes of
    both chunk halves in one gene-ordered stream (group g's shared index
    stream is applied to all 16 lanes; each node is valid on its half's
    8 lanes, junk elsewhere is excluded by the sel combine).  A DVE
    tensor_tensor_scan (bf16 in, fp32 state+out) forms prefix sums; a
    second indirect_copy extracts prefixes at the A-end and B-end gene
    boundaries; one adjacent-difference (bf16 out) gives per-(half,gene,
    batch) partials.
  - Per pass, PE matmuls against 0/1 lane-selection columns accumulate
    the valid-lane sums directly in a PSUM tile across all 4 passes
    (start on pass 0, stop on pass 3) - no SBUF accumulator.
  - PE matmul with the core's W1 shard (bf16, host-permuted, prefetched
    one group per pass) -> partial h1 [8, 1024]; host sums the 8
    partials and runs the tiny MLP tail (0.01% of FLOPs).
Emission is software-pipelined (scan_p ahead of table/gather p+1, Pool
runs gather(p+1) between gather(p) and extract(p)) so Pool stays busy
back-to-back; DVE/Act/PE/DMA all fit under Pool's ~22.4us per pass.
"""

import numpy as np

B = 8
N_SNPS = 500000
N_NODES = 2000000
N_GENES = 20000
N_FILT = 8
N_CORES = 8
BN_EPS = 1e-5

_P = 128
_NCHUNK = 64  # SNP chunks
_NTAB = 4  # table passes
_EPAD = 16


def make_cfg(n_snps, n_genes, n_cores, chunk, d1, J):
    snp_pad = _NCHUNK * chunk
    assert snp_pad >= n_snps
    assert J % 16 == 0
    gpc = n_genes // n_cores
    jt = -(-gpc // _P)
    gpad_ = jt * _P
    ns = gpad_ + gpc + 1  # boundaries: dummy + gpad A-ends + gpc B-ends
    nspad = -(-ns // _EPAD) * _EPAD
    return dict(
        n_snps=n_snps, snp_pad=snp_pad, chunk=chunk,
        n_genes=n_genes, n_cores=n_cores, gpc=gpc, gpad=jt * _P, jt=jt,
        d1=d1, J=J, ns=ns, nspad=nspad,
    )


def full_cfg(J):
    return make_cfg(N_SNPS, N_GENES, N_CORES, 8192, 1024, J)


# ---------------------------------------------------------------- device program
def build_program(cfg):
    import concourse.bass as bass
    import concourse.bacc as bacc
    import concourse.mybir as mybir
    import concourse.tile as tile

    fp32 = mybir.dt.float32
    bf16 = mybir.dt.bfloat16
    u16 = mybir.dt.uint16

    chunk, snp_pad = cfg["chunk"], cfg["snp_pad"]
    jt, d1, J = cfg["jt"], cfg["d1"], cfg["J"]
    gpad, nspad, gpc = cfg["gpad"], cfg["nspad"], cfg["gpc"]
    nd = gpad + gpc

    nc = bacc.Bacc(
        "TRN2", target_bir_lowering=False, debug=False, num_devices=cfg["n_cores"]
    )

    snp_in = nc.dram_tensor(
        "snp_perm", [_P, _NTAB * chunk], bf16, kind="ExternalInput"
    )
    filt_in = nc.dram_tensor(
        "filt_perm", [_P, _NTAB * chunk], bf16, kind="ExternalInput"
    )
    gidx_in = nc.dram_tensor(
        "gidx", [_P, _NTAB * (J // 16)], u16, kind="ExternalInput"
    )
    eidx_in = nc.dram_tensor(
        "eidx", [_P, _NTAB * (nspad // 16)], u16, kind="ExternalInput"
    )
    sel_in = nc.dram_tensor("sel", [_P, 16], bf16, kind="ExternalInput")
    route_in = nc.dram_tensor("mroute", [_P, _P], bf16, kind="ExternalInput")
    w1_in = nc.dram_tensor("w1c", [_P, jt * d1], bf16, kind="ExternalInput")
    h1_out = nc.dram_tensor("h1p", [B, d1], fp32, kind="ExternalOutput")

    rc = 512
    nblk = chunk // rc
    nhv = 4
    fhalf = chunk // nhv
    wgrp = 5 if jt % 5 == 0 else 1  # K-tiles per W1 load
    nwld = jt // wgrp

    with tile.TileContext(nc) as tc:
        with (
            tc.tile_pool(name="per", bufs=1) as perpool,
            tc.tile_pool(name="tab", bufs=2) as tabpool,
            tc.tile_pool(name="ft", bufs=2) as ftpool,
            tc.tile_pool(name="fb", bufs=3) as fbpool,
            tc.tile_pool(name="gs", bufs=2) as gspool,
            tc.tile_pool(name="qq", bufs=2) as qpool,
            tc.tile_pool(name="ex", bufs=1) as expool,
            tc.tile_pool(name="dd", bufs=1) as ddpool,
            tc.tile_pool(name="w1", bufs=3) as w1pool,
            tc.tile_pool(name="pr", bufs=3, space="PSUM") as prpool,
            tc.tile_pool(name="pst", bufs=1, space="PSUM") as pstpool,
            tc.tile_pool(name="psw", bufs=1, space="PSUM") as pswpool,
        ):
            route = perpool.tile([_P, _P], bf16, tag="route")
            nc.sync.dma_start(route[:], route_in.ap())
            sel = perpool.tile([_P, 16], bf16, tag="sel")
            nc.sync.dma_start(sel[:], sel_in.ap())
            zs = perpool.tile([_P, 1], fp32, tag="zs")
            nc.vector.memset(zs[:], 0.0)

            # adjacent-difference output; pad cols [nd, 2*gpad) stay zero
            dd = ddpool.tile([_P, 2 * gpad], bf16, tag="dd", name="dd")
            if 2 * gpad > nd:
                nc.vector.memset(dd[:, nd:], 0.0)

            # PSUM accumulator for sample_h partials [gene-tile, (t, b)]
            pst = pstpool.tile([_P, jt * B], fp32, tag="pst", name="pst")

            vtabs = {}
            last_mul = {}

            def emit_table(T):
                vtab = tabpool.tile([_P, chunk], bf16, tag="vtab", name=f"vtab{T}")
                for hv in range(nhv):
                    ft = ftpool.tile(
                        [_P, fhalf], bf16, tag="ftl", name=f"ftl{T}_{hv}"
                    )
                    nc.sync.dma_start(
                        ft[:],
                        filt_in.ap()[:, T * chunk + hv * fhalf :
                                     T * chunk + (hv + 1) * fhalf],
                    )
                    nc.sync.dma_start(
                        vtab[:, hv * fhalf : (hv + 1) * fhalf],
                        snp_in.ap()[:, T * chunk + hv * fhalf :
                                    T * chunk + (hv + 1) * fhalf],
                    )
                    for blk in range(nblk // nhv):
                        pr = prpool.tile([_P, rc], fp32, tag="pr", name="pr")
                        nc.tensor.matmul(
                            pr[:], route[:], ft[:, blk * rc : (blk + 1) * rc],
                            start=True, stop=True,
                        )
                        fb = fbpool.tile([_P, rc], bf16, tag="fb", name="fb")
                        nc.scalar.copy(fb[:], pr[:])
                        ks = slice(hv * fhalf + blk * rc,
                                   hv * fhalf + (blk + 1) * rc)
                        last_mul[T] = nc.vector.tensor_mul(
                            vtab[:, ks], vtab[:, ks], fb[:]
                        )
                vtabs[T] = vtab

            def emit_gidx_load(p):
                gidx = gspool.tile(
                    [_P, J // 16], u16, tag="gidx", name=f"gidx{p}"
                )
                nc.sync.dma_start(
                    gidx[:],
                    gidx_in.ap()[:, p * (J // 16) : (p + 1) * (J // 16)],
                )
                return gidx

            def emit_eidx_load(p):
                eidx = gspool.tile(
                    [_P, nspad // 16], u16, tag="eidx", name=f"eidx{p}"
                )
                nc.sync.dma_start(
                    eidx[:],
                    eidx_in.ap()[:, p * (nspad // 16) : (p + 1) * (nspad // 16)],
                )
                return eidx

            gather_insts = {}

            def emit_gather(p, gidx):
                gout = gspool.tile([_P, J], bf16, tag="gout", name=f"gout{p}")
                gi = nc.gpsimd.indirect_copy(gout[:], vtabs[p][:], gidx[:], True)
                gather_insts[p] = gi
                return gout

            def emit_scan(p, gout):
                q = qpool.tile([_P, J], fp32, tag="q", name=f"q{p}")
                zbc = bass.AP(zs.tensor, zs[:].offset, [zs[:].ap[0], [0, J]])
                si = nc.vector.tensor_tensor_scan(
                    q[:], zbc, gout[:], 0.0,
                    op0=mybir.AluOpType.add, op1=mybir.AluOpType.add,
                )
                # DVE is in-order: keep next table's muls ahead of the scan
                if p + 1 in last_mul:
                    bass._add_dep_helper(
                        si.ins, last_mul[p + 1].ins, sync=True,
                        reason="pipeline: scan after next table build",
                    )
                return q

            def emit_extract(p, q, eidx):
                ex = expool.tile([_P, nspad], fp32, tag="ex", name=f"ex{p}")
                ei = nc.gpsimd.indirect_copy(ex[:], q[:], eidx[:], True)
                # keep Pool busy: extract p must not preempt gather p+1
                if p + 1 in gather_insts:
                    bass._add_dep_helper(
                        ei.ins, gather_insts[p + 1].ins, sync=True,
                        reason="pipeline: extract after next gather",
                    )
                return ex

            def emit_subs(p, ex):
                # E = [Q0, A-ends (gpad, padded), B-ends (gpc)]; adjacent
                # diffs give ddA at [0,gpad) and ddB at [gpad, gpad+gpc)
                nc.vector.tensor_sub(dd[:, :gpad], ex[:, 1 : gpad + 1],
                                     ex[:, :gpad])
                for t in range(jt):
                    nc.tensor.matmul(
                        pst[:, t * B : (t + 1) * B],
                        dd[:, t * _P : (t + 1) * _P],
                        sel[:, :8],
                        start=(p == 0), stop=False,
                    )
                nc.vector.tensor_sub(dd[:, gpad : nd], ex[:, gpad + 1 : nd + 1],
                                     ex[:, gpad : nd])
                for t in range(jt):
                    nc.tensor.matmul(
                        pst[:, t * B : (t + 1) * B],
                        dd[:, gpad + t * _P : gpad + (t + 1) * _P],
                        sel[:, 8:],
                        start=False, stop=(p == _NTAB - 1),
                    )

            w1ts = []

            def emit_w1_load(jg):
                w1t = w1pool.tile([_P, wgrp * d1], bf16, tag="w1t",
                                  name=f"w1t{jg}")
                nc.sync.dma_start(
                    w1t[:],
                    w1_in.ap()[:, jg * wgrp * d1 : (jg + 1) * wgrp * d1],
                )
                w1ts.append(w1t)

            # ---- software-pipelined emission ------------------------------
            # Pool order: g0, g1, e0, g2, e1, g3, e2, e3 (gather p+1 slots
            # between extract p-1 and extract p so the scan hides behind it).
            # DVE order per cycle: muls(p+1), scan(p), subs(p-1).
            gidxs = {0: emit_gidx_load(0)}
            eidxs = {0: emit_eidx_load(0)}
            emit_table(0)
            gouts = {0: emit_gather(0, gidxs.pop(0))}
            qs, exs = {}, {}
            for p in range(_NTAB):
                if p + 1 < _NTAB:
                    gidxs[p + 1] = emit_gidx_load(p + 1)
                    eidxs[p + 1] = emit_eidx_load(p + 1)
                    emit_table(p + 1)
                if p > 0:
                    exs[p - 1] = emit_extract(p - 1, qs.pop(p - 1),
                                              eidxs.pop(p - 1))
                qs[p] = emit_scan(p, gouts.pop(p))
                if p + 1 < _NTAB:
                    gouts[p + 1] = emit_gather(p + 1, gidxs.pop(p + 1))
                if p > 0:
                    emit_subs(p - 1, exs.pop(p - 1))
                if p < nwld:
                    emit_w1_load(p)
            pl = _NTAB - 1
            exs[pl] = emit_extract(pl, qs.pop(pl), eidxs.pop(pl))
            emit_subs(pl, exs.pop(pl))

            shb = perpool.tile([_P, jt * B], bf16, tag="shb")
            nc.vector.tensor_copy(shb[:], pst[:])

            # ---- W1 matmul: accumulate over jt K-tiles --------------------
            n_half = min(512, d1)
            n_banks = -(-d1 // n_half)
            pss = []
            for nb in range(n_banks):
                psw = pswpool.tile([_P, n_half], fp32, tag=f"ps{nb}",
                                   name=f"ps{nb}")
                pss.append(psw)
            for jg in range(nwld):
                w1t = w1ts[jg]
                for jl in range(wgrp):
                    j = jg * wgrp + jl
                    lhsT = shb[:, j * B : (j + 1) * B]
                    for nb in range(n_banks):
                        nc.tensor.matmul(
                            pss[nb][:B, :],
                            lhsT,
                            w1t[:, jl * d1 + nb * n_half : jl * d1 + (nb + 1) * n_half],
                            start=(j == 0),
                            stop=(j == jt - 1),
                        )

            h1 = perpool.tile([B, d1], fp32, tag="h1")
            for nb in range(n_banks):
                nc.scalar.copy(h1[:, nb * n_half : (nb + 1) * n_half],
                               pss[nb][:B, :])
            nc.sync.dma_start(h1_out.ap(), h1[:])

    nc.compile()
    return nc


# ---------------------------------------------------------------- host side
def _wrap16(streams, dtype):
    """[8, J] per-group streams -> [128, J//16] wrapped-16 layout."""
    ngrp, J = streams.shape
    assert ngrp == 8 and J % 16 == 0
    out = np.zeros((_P, J // 16), dtype)
    for g in range(8):
        out[g * 16 : (g + 1) * 16, :] = streams[g].reshape(J // 16, 16).T
    return out


def _chunk_placement(snp_ids, node_seg, chunk):
    """Per-core chunk->slot placement balancing merged A+B bucket sizes.

    Returns (J, [C_0..C_7]) where C_c[T][j] is the chunk held by slot j
    (j = g + 8h) of pass T on core c.
    """
    ids = np.asarray(snp_ids).astype(np.int64)
    seg = np.asarray(node_seg).astype(np.int64)
    gpc = N_GENES // N_CORES
    gene_starts = np.searchsorted(seg, np.arange(0, N_GENES + 1, gpc))
    Cs = []
    mx = 0
    for c in range(N_CORES):
        lo, hi = gene_starts[c], gene_starts[c + 1]
        cnt = np.bincount(ids[lo:hi] // chunk, minlength=_NCHUNK)
        order = np.argsort(cnt)[::-1]  # heavy..light
        C = np.zeros((_NTAB, 16), np.int64)
        for i in range(32):
            a, b = order[i], order[63 - i]
            T, g = i % _NTAB, i // _NTAB % 8
            C[T][g] = a
            C[T][8 + g] = b
            mx = max(mx, int(cnt[a] + cnt[b]))
        Cs.append(C)
    J = -(-(mx + 1) // 16) * 16
    assert J <= 65535, f"pass stream length {J} exceeds uint16 index range"
    return J, Cs


def prep_inputs(cfg, snp, snp_ids, node_seg, filters, W1, Cs):
    """Index/metadata preprocessing + zero-padding + pure layout permutation
    and bf16 casting; all value computation happens on device."""
    import ml_dtypes

    snp_pad_n, chunk = cfg["snp_pad"], cfg["chunk"]
    gpc, gpad, d1 = cfg["gpc"], cfg["gpad"], cfg["d1"]
    n_genes, n_snps = cfg["n_genes"], cfg["n_snps"]
    J, nspad = cfg["J"], cfg["nspad"]
    n_cores = cfg["n_cores"]
    bf = ml_dtypes.bfloat16

    snp_p = np.zeros((B, snp_pad_n), bf)
    snp_p[:, :n_snps] = np.asarray(snp, np.float32).astype(bf)
    filt_p = np.zeros((N_FILT, snp_pad_n), bf)
    filt_p[:, :n_snps] = np.asarray(filters, np.float32).astype(bf)
    snp_ch = snp_p.reshape(B, _NCHUNK, chunk)
    filt_ch = filt_p.reshape(N_FILT, _NCHUNK, chunk)

    # mean+replicate routing: out[m, j] = (1/8) sum_r filt_perm[8*slot(m)+r, j]
    # where slot(m) = m//16 + 8*((m%16)//8)
    mroute = np.zeros((_P, _P), bf)
    for mm in range(_P):
        spt = mm // 16 + 8 * ((mm % 16) // 8)
        mroute[spt * 8 : spt * 8 + 8, mm] = 1.0 / N_FILT

    sel = np.zeros((_P, 16), bf)
    for p in range(_P):
        sel[p, p % 16] = 1.0

    ids = np.asarray(snp_ids).astype(np.int64)
    seg = np.asarray(node_seg).astype(np.int64)
    gene_starts = np.searchsorted(seg, np.arange(0, n_genes + 1))
    node_chunk = ids // chunk
    node_lidx = (ids % chunk).astype(np.uint16)

    W1f = np.asarray(W1, np.float32)
    per_core = []
    for c in range(n_cores):
        C = Cs[c]
        # snp_perm: row p=16g+8h+b, pass-T cols hold snp[b, C[T][g+8h]-chunk]
        # filt_perm: row q holds filters[q%8, C[T][q//8]-chunk]
        snp_perm = np.empty((_P, _NTAB * chunk), bf)
        filt_perm = np.empty((_P, _NTAB * chunk), bf)
        for T in range(_NTAB):
            for g in range(8):
                for h in range(2):
                    ch = C[T][g + 8 * h]
                    rows = slice(16 * g + 8 * h, 16 * g + 8 * h + 8)
                    snp_perm[rows, T * chunk : (T + 1) * chunk] = snp_ch[:, ch, :]
            for j in range(16):
                ch = C[T][j]
                filt_perm[8 * j : 8 * j + 8, T * chunk : (T + 1) * chunk] = (
                    filt_ch[:, ch, :]
                )

        lo, hi = gene_starts[c * gpc], gene_starts[(c + 1) * gpc]
        cid_chunk = node_chunk[lo:hi]
        cid_lidx = node_lidx[lo:hi]
        cid_gene = seg[lo:hi] - c * gpc  # local gene, sorted ascending

        gidx = np.zeros((_NTAB, 8, J), np.uint16)
        eidx = np.zeros((_NTAB, 8, nspad), np.uint16)
        for T in range(_NTAB):
            for g_ in range(8):
                chA, chB = C[T][g_], C[T][8 + g_]
                mA = cid_chunk == chA
                mB = cid_chunk == chB
                lidxA, lgeneA = cid_lidx[mA], cid_gene[mA]
                lidxB, lgeneB = cid_lidx[mB], cid_gene[mB]
                cntA, cntB = len(lidxA), len(lidxB)
                assert cntA + cntB + 1 <= J, f"bucket {cntA+cntB} exceeds J={J}"
                # merged stream: [dummy, chunk-A nodes by gene, chunk-B nodes]
                gidx[T, g_, 1 : 1 + cntA] = lidxA
                gidx[T, g_, 1 + cntA : 1 + cntA + cntB] = lidxB
                # boundary positions: [0, A-ends (gpad, pad=end-of-A), B-ends]
                FA = np.searchsorted(lgeneA, np.arange(1, gpc + 1))
                FB = cntA + np.searchsorted(lgeneB, np.arange(1, gpc + 1))
                pos = np.zeros(nspad, np.int64)
                pos[1 : 1 + gpc] = FA
                pos[1 + gpc : 1 + gpad] = FA[-1] if gpc else 0
                pos[1 + gpad : 1 + gpad + gpc] = FB
                pos[1 + gpad + gpc :] = FB[-1]
                eidx[T, g_] = pos.astype(np.uint16)

        w1c = np.zeros((gpad, d1), np.float32)
        w1c[:gpc] = W1f[c * gpc : (c + 1) * gpc]
        jt_ = gpad // _P
        w1perm = np.ascontiguousarray(
            w1c.reshape(jt_, _P, d1).transpose(1, 0, 2).reshape(_P, jt_ * d1)
        ).astype(bf)
        gidx_all = np.concatenate(
            [_wrap16(gidx[p], np.uint16) for p in range(_NTAB)], axis=1
        )
        eidx_all = np.concatenate(
            [_wrap16(eidx[p], np.uint16) for p in range(_NTAB)], axis=1
        )
        core_map = dict(
            snp_perm=snp_perm, filt_perm=filt_perm, sel=sel, w1c=w1perm,
            mroute=mroute, gidx=gidx_all, eidx=eidx_all,
        )
        per_core.append(core_map)
    return per_core


def host_tail(h1_sum, b1, g1, be1, W2, b2, g2, be2, W3, b3, g3, be3,
              Wh1, bh1, gh, beh, Wh2, bh2):
    def bn(x, g, be):
        return x * (g / np.sqrt(np.float32(1.0 + BN_EPS))) + be

    relu = lambda x: np.maximum(x, np.float32(0.0))
    h = relu(bn(h1_sum + b1, g1, be1))
    h = relu(bn(h @ W2 + b2, g2, be2))
    feat = relu(bn(h @ W3 + b3, g3, be3))
    m = relu(bn(feat[:, :15] @ Wh1 + bh1, gh, beh))
    return (m @ Wh2 + bh2).astype(np.float32)


_CACHE = {}


def kernel(snp, snp_ids, node_seg, filters, W1, b1, g1, be1, W2, b2, g2, be2,
           W3, b3, g3, be3, Wh1, bh1, gh, beh, Wh2, bh2):
    from concourse import bass_utils

    J, Cs = _chunk_placement(snp_ids, node_seg, 8192)
    cfg = full_cfg(J)

    key = ("full", J)
    if key not in _CACHE:
        _CACHE[key] = build_program(cfg)
    nc = _CACHE[key]

    in_maps = prep_inputs(cfg, snp, snp_ids, node_seg, filters, W1, Cs)
    res = bass_utils.run_bass_kernel_spmd(
        nc, in_maps, core_ids=list(range(cfg["n_cores"]))
    )
    h1_sum = np.zeros((B, cfg["d1"]), np.float32)
    for c in range(cfg["n_cores"]):
        h1_sum += res.results[c]["h1p"]

    f32 = lambda x: np.asarray(x, np.float32)
    return host_tail(h1_sum, f32(b1), f32(g1), f32(be1), f32(W2), f32(b2),
                     f32(g2), f32(be2), f32(W3), f32(b3), f32(g3), f32(be3),
                     f32(Wh1), f32(bh1), f32(gh), f32(beh), f32(Wh2), f32(bh2))
